# revision 1
# baseline (speedup 1.0000x reference)
"""Trainium2 Bass kernel for CrossAttentionConditionInjection.

Math note: in the reference, K and V are projections of a single per-batch
condition vector broadcast identically across all S key positions.  The
attention scores are therefore constant along the softmax axis, softmax is
exactly uniform (1/S each), and the attention output is the mean of S
identical V rows, i.e. V itself.  The whole module collapses exactly to

    out[b, s, :] = (condition[b] @ Wv.T + bv) @ Wo.T + bo      (for every s)

independent of hidden_states / Wq / bq / Wk / bk.  (S = 1024 is a power of
two, so even the fp32 softmax-average path is bit-exact against this.)

Device strategy (8 NeuronCores on one trn2 chip, SPMD, two small NEFFs —
a collective-based single NEFF was measured slower: any collective costs
~80us wall in this runtime, while a whole no-collective NEFF is ~12us):

  Launch A: Wv.T column-sharded 8x.  Core i computes
            vT[256i:256(i+1), :] = (condition @ Wv.T + bv).T[shard]
            and returns the (256, 4) shard.  Host concatenates to the
            full (2048, 4) vT (layout only).
  Launch B: Wo.T column-sharded 8x.  Core i computes
            r[:, shard] = vT.T @ Wo.T[:, shard], folds bo + the
            broadcast over sequence positions into one selector matmul
            per batch entry, and writes its (4, 1024, 256) output
            slice.  Host concatenates along channels (layout only).

Both launches are Tile kernels (USE_RAW=False): a raw-bass rewrite with
manual semaphores was measured slower (90us vs 82us) — Tile's per-chunk
DMA/compute pipelining beats its ~8us/NEFF barrier overhead here.
"""

import numpy as np

import concourse.bass as bass
import concourse.mybir as mybir
import concourse.tile as tile
from concourse import bacc
from concourse.bass_utils import run_bass_kernel_spmd
from concourse.masks import make_identity

B = 4
S = 1024
D = 2048
N_CORES = 8
JC = D // N_CORES  # 256 channels per core (v-shard in A, out-shard in B)
P = 128
KT = D // P  # 16 k-chunks
FP = mybir.dt.float32

USE_RAW = False

N_WARM = 8  # junk matmuls to lift the PE HAM clock gate while DMAs stream


def _new_nc():
    return bacc.Bacc(
        "TRN2",
        target_bir_lowering=False,
        debug=False,
        enable_asserts=False,
        num_devices=N_CORES,
    )


def build_nc_a_raw():
    nc = _new_nc()
    ct_d = nc.dram_tensor("ct", [D, B], FP, kind="ExternalInput").ap()
    wv_d = nc.dram_tensor("wv_s", [D, JC], FP, kind="ExternalInput").ap()
    bv_d = nc.dram_tensor("bv_s", [P, JC // P], FP, kind="ExternalInput").ap()
    id4_d = nc.dram_tensor("id4", [B, B], FP, kind="ExternalInput").ap()
    vt_d = nc.dram_tensor("vt_s", [JC, B], FP, kind="ExternalOutput").ap()

    N_IN = 3 + KT  # ct, bv, id4, wv x16

    with (
        nc.semaphore("s_in") as s_in,
        nc.semaphore("s_h0") as s_h0,
        nc.semaphore("s_h1") as s_h1,
        nc.semaphore("s_wu") as s_wu,
        nc.semaphore("s_pv") as s_pv,
        nc.semaphore("s_vl") as s_vl,
        nc.semaphore("s_mm") as s_mm,
        nc.semaphore("s_vt") as s_vt,
        nc.semaphore("s_out") as s_out,
        nc.sbuf_tensor("ct_sb", [P, KT * B], FP) as ct_sb,
        nc.sbuf_tensor("wv_sb", [P, KT * JC], FP) as wv_sb,
        nc.sbuf_tensor("bv_sb", [P, JC // P], FP) as bv_sb,
        nc.sbuf_tensor("vl_sb", [B, JC], FP) as vl_sb,
        nc.sbuf_tensor("vtl_sb", [P, (JC // P) * B], FP) as vtl_sb,
        nc.sbuf_tensor("id4_sb", [B, B], FP) as id4_sb,
        nc.sbuf_tensor("wup_sb", [P, P], FP) as wup_sb,
        nc.psum_tensor("pwu", [P, 512], FP) as pwu,
        nc.psum_tensor("pv", [B, 512], FP) as pv,
        nc.psum_tensor("pt0", [P, 512], FP) as pt0,
        nc.psum_tensor("pt1", [P, 512], FP) as pt1,
        nc.Block() as block,
    ):

        @block.sync
        def _(sync):
            sync.dma_start(id4_sb[:, :], id4_d[:, :]).then_inc(s_in, 16)
            sync.dma_start(
                ct_sb[:, :].rearrange("p (t b) -> p t b", t=KT),
                ct_d.rearrange("(t p) b -> p t b", p=P),
            ).then_inc(s_in, 16)
            sync.dma_start(bv_sb[:, :], bv_d[:, :]).then_inc(s_in, 16)
            for t in range(KT):
                sync.dma_start(
                    wv_sb[:, t * JC : (t + 1) * JC], wv_d[t * P : (t + 1) * P, :]
                ).then_inc(s_h0 if t < KT // 2 else s_h1, 16)
            sync.wait_ge(s_vt, 2)
            sync.dma_start(
                vt_d.rearrange("(g p) b -> p g b", p=P),
                vtl_sb[:, :].rearrange("p (g b) -> p g b", g=JC // P),
            ).then_inc(s_out, 16)
            sync.wait_ge(s_out, 16)

        @block.vector
        def _(vector):
            vector.memset(wup_sb[:, :], 0.0).then_inc(s_wu, 1)
            vector.wait_ge(s_pv, 1)
            vector.tensor_copy(vl_sb[:, :], pv[:, 0:JC]).then_inc(s_vl, 1)
            for g in range(JC // P):
                pt = pt0 if g == 0 else pt1
                vector.wait_ge(s_mm, g + 1)
                vector.tensor_scalar_add(
                    vtl_sb[:, g * B : (g + 1) * B], pt[:, 0:B], bv_sb[:, g : g + 1]
                ).then_inc(s_vt, 1)

        @block.tensor
        def _(tensor):
            tensor.wait_ge(s_wu, 1)
            for w in range(N_WARM):
                tensor.matmul(
                    pwu[:, 0:P], wup_sb[:, :], wup_sb[:, :], start=True, stop=True
                )
            tensor.wait_ge(s_in, 3 * 16)
            tensor.wait_ge(s_h0, (KT // 2) * 16)
            for t in range(KT):
                if t == KT // 2:
                    tensor.wait_ge(s_h1, (KT // 2) * 16)
                mm = tensor.matmul(
                    pv[:, 0:JC],
                    ct_sb[:, t * B : (t + 1) * B],
                    wv_sb[:, t * JC : (t + 1) * JC],
                    start=(t == 0),
                    stop=(t == KT - 1),
                )
            mm.then_inc(s_pv, 1)
            tensor.wait_ge(s_vl, 1)
            for g in range(JC // P):
                pt = pt0 if g == 0 else pt1
                tensor.transpose(
                    pt[:, 0:B], vl_sb[:, g * P : (g + 1) * P], id4_sb[:, :]
                ).then_inc(s_mm, 1)

    nc.compile()
    return nc


def build_nc_b_raw():
    nc = _new_nc()
    vt_d = nc.dram_tensor("vt", [D, B], FP, kind="ExternalInput").ap()
    wo_d = nc.dram_tensor("wo_s", [D, JC], FP, kind="ExternalInput").ap()
    bo_d = nc.dram_tensor("bo_s", [1, JC], FP, kind="ExternalInput").ap()
    sel_d = nc.dram_tensor("sel", [B + 1, B * P], FP, kind="ExternalInput").ap()
    out_d = nc.dram_tensor("out", [B, S, JC], FP, kind="ExternalOutput").ap()

    N_IN = 3 + KT  # vt, bo, sel, wo x16

    with (
        nc.semaphore("s_in") as s_in,
        nc.semaphore("s_h0") as s_h0,
        nc.semaphore("s_h1") as s_h1,
        nc.semaphore("s_wu") as s_wu,
        nc.semaphore("s_r") as s_r,
        nc.semaphore("s_rb") as s_rb,
        nc.semaphore("s_bct") as s_bct,
        nc.semaphore("s_bc") as s_bc,
        nc.semaphore("s_out") as s_out,
        nc.sbuf_tensor("vt_sb", [P, KT * B], FP) as vt_sb,
        nc.sbuf_tensor("wo_sb", [P, KT * JC], FP) as wo_sb,
        nc.sbuf_tensor("rb_sb", [B + 1, JC], FP) as rb_sb,
        nc.sbuf_tensor("sel_sb", [B + 1, B * P], FP) as sel_sb,
        nc.sbuf_tensor("bc_sb", [P, B * JC], FP) as bc_sb,
        nc.sbuf_tensor("wup_sb", [P, P], FP) as wup_sb,
        nc.psum_tensor("pwu", [P, 512], FP) as pwu,
        nc.psum_tensor("pr", [B, 512], FP) as pr,
        nc.psum_tensor("pb0", [P, 512], FP) as pb0,
        nc.psum_tensor("pb1", [P, 512], FP) as pb1,
        nc.Block() as block,
    ):

        @block.sync
        def _(sync):
            sync.dma_start(
                vt_sb[:, :].rearrange("p (g b) -> p g b", g=KT),
                vt_d.rearrange("(g p) b -> p g b", p=P),
            ).then_inc(s_in, 16)
            sync.dma_start(rb_sb[B : B + 1, :], bo_d[:, :]).then_inc(s_in, 16)
            sync.dma_start(sel_sb[:, :], sel_d[:, :]).then_inc(s_in, 16)
            for g in range(KT):
                sync.dma_start(
                    wo_sb[:, g * JC : (g + 1) * JC], wo_d[g * P : (g + 1) * P, :]
                ).then_inc(s_h0 if g < KT // 2 else s_h1, 16)
            for b in range(B):
                sync.wait_ge(s_bc, b + 1)
                for sc in range(S // P):
                    sync.dma_start(
                        out_d[b, sc * P : (sc + 1) * P, :],
                        bc_sb[:, b * JC : (b + 1) * JC],
                    ).then_inc(s_out, 16)
            sync.wait_ge(s_out, B * (S // P) * 16)

        @block.vector
        def _(vector):
            vector.memset(wup_sb[:, :], 0.0).then_inc(s_wu, 1)
            vector.wait_ge(s_r, 1)
            vector.tensor_copy(rb_sb[0:B, :], pr[:, 0:JC]).then_inc(s_rb, 1)
            for b in range(B):
                pb = pb0 if b % 2 == 0 else pb1
                vector.wait_ge(s_bct, b + 1)
                vector.tensor_copy(
                    bc_sb[:, b * JC : (b + 1) * JC], pb[:, 0:JC]
                ).then_inc(s_bc, 1)

        @block.tensor
        def _(tensor):
            tensor.wait_ge(s_wu, 1)
            for w in range(N_WARM):
                tensor.matmul(
                    pwu[:, 0:P], wup_sb[:, :], wup_sb[:, :], start=True, stop=True
                )
            tensor.wait_ge(s_in, 3 * 16)
            tensor.wait_ge(s_h0, (KT // 2) * 16)
            for g in range(KT):
                if g == KT // 2:
                    tensor.wait_ge(s_h1, (KT // 2) * 16)
                mm = tensor.matmul(
                    pr[:, 0:JC],
                    vt_sb[:, g * B : (g + 1) * B],
                    wo_sb[:, g * JC : (g + 1) * JC],
                    start=(g == 0),
                    stop=(g == KT - 1),
                )
            mm.then_inc(s_r, 1)
            tensor.wait_ge(s_rb, 1)
            for b in range(B):
                pb = pb0 if b % 2 == 0 else pb1
                if b >= 2:
                    tensor.wait_ge(s_bc, b - 1)
                tensor.matmul(
                    pb[:, 0:JC],
                    sel_sb[:, b * P : (b + 1) * P],
                    rb_sb[:, :],
                    start=True,
                    stop=True,
                ).then_inc(s_bct, 1)

    nc.compile()
    return nc


def build_nc_a_tile():
    nc = _new_nc()
    ct_d = nc.dram_tensor("ct", [D, B], FP, kind="ExternalInput").ap()
    wv_d = nc.dram_tensor("wv_s", [D, JC], FP, kind="ExternalInput").ap()
    bv_d = nc.dram_tensor("bv_s", [P, JC // P], FP, kind="ExternalInput").ap()
    id4_d = nc.dram_tensor("id4", [B, B], FP, kind="ExternalInput").ap()
    vt_d = nc.dram_tensor("vt_s", [JC, B], FP, kind="ExternalOutput").ap()

    with tile.TileContext(nc) as tc:
        with (
            tc.tile_pool(name="work", bufs=1) as work,
            tc.tile_pool(name="pv", bufs=1, space="PSUM") as pv_pool,
            tc.tile_pool(name="pt", bufs=2, space="PSUM") as pt_pool,
        ):
            wv_sb = work.tile([P, KT, JC], FP)
            ct_sb = work.tile([P, KT, B], FP)
            bv_sb = work.tile([P, JC // P], FP)
            vl_sb = work.tile([B, JC], FP)
            vtl_sb = work.tile([P, JC // P, B], FP)
            id4_sb = work.tile([B, B], FP)
            nc.sync.dma_start(id4_sb[:, :], id4_d[:, :])

            nc.sync.dma_start(ct_sb[:, :, :], ct_d.rearrange("(t p) b -> p t b", p=P))
            for t in range(KT):
                nc.sync.dma_start(wv_sb[:, t, :], wv_d[t * P : (t + 1) * P, :])
            nc.sync.dma_start(bv_sb[:, :], bv_d[:, :])

            pv = pv_pool.tile([B, JC], FP)
            for t in range(KT):
                nc.tensor.matmul(
                    pv[:, :],
                    ct_sb[:, t, :],
                    wv_sb[:, t, :],
                    start=(t == 0),
                    stop=(t == KT - 1),
                )
            nc.vector.tensor_copy(vl_sb[:, :], pv[:, :])

            for g in range(JC // P):
                pt = pt_pool.tile([P, B], FP)
                nc.tensor.transpose(
                    pt[:, :], vl_sb[:, g * P : (g + 1) * P], id4_sb[:, :]
                )
                nc.vector.tensor_scalar_add(
                    vtl_sb[:, g, :], pt[:, :], bv_sb[:, g : g + 1]
                )
            nc.sync.dma_start(
                vt_d.rearrange("(g p) b -> p g b", p=P), vtl_sb[:, :, :]
            )

    nc.compile()
    return nc


def build_nc_b_tile():
    nc = _new_nc()
    vt_d = nc.dram_tensor("vt", [D, B], FP, kind="ExternalInput").ap()
    wo_d = nc.dram_tensor("wo_s", [D, JC], FP, kind="ExternalInput").ap()
    bo_d = nc.dram_tensor("bo_s", [1, JC], FP, kind="ExternalInput").ap()
    sel_d = nc.dram_tensor("sel", [B + 1, B * P], FP, kind="ExternalInput").ap()
    out_d = nc.dram_tensor("out", [B, S, JC], FP, kind="ExternalOutput").ap()

    with tile.TileContext(nc) as tc:
        with (
            tc.tile_pool(name="work", bufs=1) as work,
            tc.tile_pool(name="pr", bufs=1, space="PSUM") as pr_pool,
            tc.tile_pool(name="pb", bufs=2, space="PSUM") as pb_pool,
        ):
            wo_sb = work.tile([P, KT, JC], FP)
            vt_sb = work.tile([P, KT, B], FP)
            rb_sb = work.tile([B + 1, JC], FP)
            sel_sb = work.tile([B + 1, B * P], FP)
            bc_sb = work.tile([P, B, JC], FP)

            nc.sync.dma_start(vt_sb[:, :, :], vt_d.rearrange("(g p) b -> p g b", p=P))
            for g in range(KT):
                nc.sync.dma_start(wo_sb[:, g, :], wo_d[g * P : (g + 1) * P, :])
            nc.sync.dma_start(rb_sb[B : B + 1, :], bo_d[:, :])
            nc.sync.dma_start(sel_sb[:, :], sel_d[:, :])

            pr = pr_pool.tile([B, JC], FP)
            for g in range(KT):
                nc.tensor.matmul(
                    pr[:, :],
                    vt_sb[:, g, :],
                    wo_sb[:, g, :],
                    start=(g == 0),
                    stop=(g == KT - 1),
                )
            nc.vector.tensor_copy(rb_sb[0:B, :], pr[:, :])

            for b in range(B):
                pb = pb_pool.tile([P, JC], FP)
                nc.tensor.matmul(
                    pb[:, :],
                    sel_sb[:, b * P : (b + 1) * P],
                    rb_sb[:, :],
                    start=True,
                    stop=True,
                )
                nc.vector.tensor_copy(bc_sb[:, b, :], pb[:, :])
                for sc in range(S // P):
                    nc.sync.dma_start(
                        out_d[b, sc * P : (sc + 1) * P, :], bc_sb[:, b, :]
                    )

    nc.compile()
    return nc


def build_nc_a():
    return build_nc_a_raw() if USE_RAW else build_nc_a_tile()


def build_nc_b():
    return build_nc_b_raw() if USE_RAW else build_nc_b_tile()


def make_in_maps_a(condition, Wv, bv):
    ct = np.ascontiguousarray(np.asarray(condition, dtype=np.float32).T)
    wvT = np.asarray(Wv, dtype=np.float32).T
    bv = np.asarray(bv, dtype=np.float32)
    id4 = np.eye(B, dtype=np.float32)
    in_maps = []
    for i in range(N_CORES):
        sl = slice(i * JC, (i + 1) * JC)
        in_maps.append(
            {
                "ct": ct,
                "wv_s": np.ascontiguousarray(wvT[:, sl]),
                "bv_s": np.ascontiguousarray(bv[sl].reshape(JC // P, P).T),
                "id4": id4,
            }
        )
    return in_maps


def make_in_maps_b(vt, Wo, bo):
    woT = np.asarray(Wo, dtype=np.float32).T
    bo = np.asarray(bo, dtype=np.float32)
    sel = np.zeros((B + 1, B * P), dtype=np.float32)
    for b in range(B):
        sel[b, b * P : (b + 1) * P] = 1.0
    sel[B, :] = 1.0
    in_maps = []
    for i in range(N_CORES):
        sl = slice(i * JC, (i + 1) * JC)
        in_maps.append(
            {
                "vt": vt,
                "wo_s": np.ascontiguousarray(woT[:, sl]),
                "bo_s": np.ascontiguousarray(bo[sl]).reshape(1, JC),
                "sel": sel,
            }
        )
    return in_maps


_NC_CACHE = None


def get_ncs():
    global _NC_CACHE
    if _NC_CACHE is None:
        _NC_CACHE = (build_nc_a(), build_nc_b())
    return _NC_CACHE


def kernel(**inputs):
    nc_a, nc_b = get_ncs()
    cores = list(range(N_CORES))

    res_a = run_bass_kernel_spmd(
        nc_a,
        make_in_maps_a(inputs["condition"], inputs["Wv"], inputs["bv"]),
        core_ids=cores,
    )
    vt = np.ascontiguousarray(
        np.concatenate([r["vt_s"] for r in res_a.results], axis=0)
    )

    res_b = run_bass_kernel_spmd(
        nc_b,
        make_in_maps_b(vt, inputs["Wo"], inputs["bo"]),
        core_ids=cores,
    )
    out = np.concatenate([r["out"] for r in res_b.results], axis=-1)
    return out



# revision 2
# speedup vs baseline: 1.5417x; 1.5417x over previous
"""Trainium2 Bass kernel for CrossAttentionConditionInjection.

Math note: in the reference, K and V are projections of a single per-batch
condition vector broadcast identically across all S key positions.  The
attention scores are therefore constant along the softmax axis, softmax is
exactly uniform (1/S each), and the attention output is the mean of S
identical V rows, i.e. V itself.  The whole module collapses exactly to

    out[b, s, :] = (condition[b] @ Wv.T + bv) @ Wo.T + bo      (for every s)

independent of hidden_states / Wq / bq / Wk / bk.  (S = 1024 is a power of
two, so even the fp32 softmax-average path is bit-exact against this.)

Device strategy (8 NeuronCores, SPMD, two small NEFFs; collectives in this
runtime cost ~80us so the v-gather between the two matmuls happens on host):

  Launch A: Wv.T column-sharded 8x.  Core i computes
            v[:, 256i:256(i+1)] = condition @ Wv.T[:, shard] + bv[shard]
            as a (4, 256) bf16 tile.  Host concatenates + transposes
            (layout only) to the chunked vT layout launch B wants.
  Launch B: Wo.T column-sharded 8x.  Core i computes
            r[:, shard] = v @ Wo.T[:, shard] + bo[shard], broadcasts the
            4 rows to 128 partitions with one selector matmul, replicates
            each row 32x along the free axis (log2 doublings on DVE), and
            writes its (128, 8192) bf16 output block = the (4, 1024, 256)
            channel shard with 8 KB contiguous lines per partition.

Perf notes vs the first working version (80.4us):
  - everything bf16 on device (weights/intermediates/output): halves DMA
    bytes and gives 4x PE throughput; measured rel err ~3e-3 vs the 2e-2
    gate (validated in numpy with ml_dtypes).
  - weights staged host-side into partition-major [128, 4096] bf16 so each
    DMA moves 4-8 KB contiguous per partition (the old layout moved 1 KB
    lines in 16 separate DMAs -> ~450 B packets, ~170 GB/s).
  - bulk DMAs split across both HWDGE rings (sync + scalar engines).
  - output written as [128, 8192] bf16 (one 16 KB line per partition),
    reshaped back to (4, 1024, 256) on host (pure reshape, no transpose).
"""

import numpy as np
import ml_dtypes

import concourse.bass as bass
import concourse.mybir as mybir
import concourse.tile as tile
from concourse import bacc
from concourse.bass_utils import run_bass_kernel_spmd

B = 4
S = 1024
D = 2048
N_CORES = 8
JC = D // N_CORES  # 256 channels per core (v-shard in A, out-shard in B)
P = 128
KT = D // P  # 16 k-chunks
FP = mybir.dt.float32
BF = mybir.dt.bfloat16
NPBF = ml_dtypes.bfloat16

RPP = (B * S) // P  # 32 replicated rows per partition in the output block
LINE = RPP * JC  # 8192 bf16 elems per output partition line


def _new_nc():
    return bacc.Bacc(
        "TRN2",
        target_bir_lowering=False,
        debug=False,
        enable_asserts=False,
        num_devices=N_CORES,
    )


def _chunked(a2d):
    """(D, n) -> [P, KT*n] bf16, chunk t holds rows t*128..t*128+127."""
    d, n = a2d.shape
    assert d == D
    return np.ascontiguousarray(
        a2d.astype(NPBF).reshape(KT, P, n).transpose(1, 0, 2).reshape(P, KT * n)
    )


def build_nc_a():
    nc = _new_nc()
    ct_d = nc.dram_tensor("ctb", [P, KT * B], BF, kind="ExternalInput").ap()
    wv_d = nc.dram_tensor("wvb", [P, KT * JC], BF, kind="ExternalInput").ap()
    bv_d = nc.dram_tensor("bvb", [1, JC], BF, kind="ExternalInput").ap()
    v_d = nc.dram_tensor("v_s", [B, JC], BF, kind="ExternalOutput").ap()

    H = KT // 2
    with tile.TileContext(nc) as tc:
        with (
            tc.tile_pool(name="work", bufs=1) as work,
            tc.tile_pool(name="pv", bufs=1, space="PSUM") as pv_pool,
        ):
            wv_sb = work.tile([P, KT, JC], BF)
            ct_sb = work.tile([P, KT, B], BF)
            bv_sb = work.tile([1, JC], BF)
            ones_sb = work.tile([1, B], BF)
            v_sb = work.tile([B, JC], BF)

            nc.scalar.dma_start(ct_sb[:, :, :], ct_d.rearrange("p (t b) -> p t b", t=KT))
            nc.scalar.dma_start(bv_sb[:, :], bv_d[:, :])
            nc.vector.memset(ones_sb[:, :], 1.0)
            nc.sync.dma_start(
                wv_sb[:, 0:H, :],
                wv_d[:, 0 : H * JC].rearrange("p (t j) -> p t j", t=H),
            )
            nc.scalar.dma_start(
                wv_sb[:, H:KT, :],
                wv_d[:, H * JC : KT * JC].rearrange("p (t j) -> p t j", t=H),
            )

            pv = pv_pool.tile([B, JC], FP)
            for t in range(KT):
                nc.tensor.matmul(
                    pv[:, :],
                    ct_sb[:, t, :],
                    wv_sb[:, t, :],
                    start=(t == 0),
                    stop=False,
                )
            nc.tensor.matmul(
                pv[:, :], ones_sb[:, :], bv_sb[:, :], start=False, stop=True
            )
            nc.vector.tensor_copy(v_sb[:, :], pv[:, :])
            nc.sync.dma_start(v_d[:, :], v_sb[:, :])

    nc.compile()
    return nc


def build_nc_b():
    nc = _new_nc()
    vt_d = nc.dram_tensor("vtb", [P, KT * B], BF, kind="ExternalInput").ap()
    wo_d = nc.dram_tensor("wob", [P, KT * JC], BF, kind="ExternalInput").ap()
    bo_d = nc.dram_tensor("bob", [1, JC], BF, kind="ExternalInput").ap()
    sel_d = nc.dram_tensor("selb", [B + 1, P], BF, kind="ExternalInput").ap()
    out_d = nc.dram_tensor("out", [P, LINE], BF, kind="ExternalOutput").ap()

    H = KT // 2
    with tile.TileContext(nc) as tc:
        with (
            tc.tile_pool(name="work", bufs=1) as work,
            tc.tile_pool(name="pr", bufs=1, space="PSUM") as pr_pool,
            tc.tile_pool(name="pb", bufs=1, space="PSUM") as pb_pool,
        ):
            wo_sb = work.tile([P, KT, JC], BF)
            vt_sb = work.tile([P, KT, B], BF)
            rb_sb = work.tile([B + 1, JC], BF)
            sel_sb = work.tile([B + 1, P], BF)
            rep_sb = work.tile([P, LINE], BF)

            nc.scalar.dma_start(vt_sb[:, :, :], vt_d.rearrange("p (t b) -> p t b", t=KT))
            nc.scalar.dma_start(rb_sb[B : B + 1, :], bo_d[:, :])
            nc.scalar.dma_start(sel_sb[:, :], sel_d[:, :])
            nc.sync.dma_start(
                wo_sb[:, 0:H, :],
                wo_d[:, 0 : H * JC].rearrange("p (t j) -> p t j", t=H),
            )
            nc.scalar.dma_start(
                wo_sb[:, H:KT, :],
                wo_d[:, H * JC : KT * JC].rearrange("p (t j) -> p t j", t=H),
            )

            pr = pr_pool.tile([B, JC], FP)
            for t in range(KT):
                nc.tensor.matmul(
                    pr[:, :],
                    vt_sb[:, t, :],
                    wo_sb[:, t, :],
                    start=(t == 0),
                    stop=(t == KT - 1),
                )
            nc.vector.tensor_copy(rb_sb[0:B, :], pr[:, :])

            # one selector matmul: partition p <- row p//32 (+ bias row)
            pb = pb_pool.tile([P, JC], FP)
            nc.tensor.matmul(
                pb[:, :], sel_sb[:, :], rb_sb[:, :], start=True, stop=True
            )
            nc.vector.tensor_copy(rep_sb[:, 0:JC], pb[:, :])
            # log2 doublings to 32 copies per partition
            n = JC
            while n < LINE:
                nc.vector.tensor_copy(rep_sb[:, n : 2 * n], rep_sb[:, 0:n])
                n *= 2
            nc.sync.dma_start(out_d[:, 0 : LINE // 2], rep_sb[:, 0 : LINE // 2])
            nc.scalar.dma_start(out_d[:, LINE // 2 : LINE], rep_sb[:, LINE // 2 : LINE])

    nc.compile()
    return nc


def make_in_maps_a(condition, Wv, bv):
    ct = _chunked(np.asarray(condition, dtype=np.float32).T)  # [P, KT*B]
    wvT = np.asarray(Wv, dtype=np.float32).T
    bvb = np.asarray(bv, dtype=np.float32).astype(NPBF)
    in_maps = []
    for i in range(N_CORES):
        sl = slice(i * JC, (i + 1) * JC)
        in_maps.append(
            {
                "ctb": ct,
                "wvb": _chunked(wvT[:, sl]),
                "bvb": bvb[sl].reshape(1, JC),
            }
        )
    return in_maps


def make_in_maps_b(v_bf, Wo, bo):
    # v_bf: (B, D) bf16.  Launch B wants vT chunked [P, KT*B].
    vt = np.ascontiguousarray(
        v_bf.T.reshape(KT, P, B).transpose(1, 0, 2).reshape(P, KT * B)
    )
    woT = np.asarray(Wo, dtype=np.float32).T
    bob = np.asarray(bo, dtype=np.float32).astype(NPBF)
    sel = np.zeros((B + 1, P), dtype=NPBF)
    for b in range(B):
        sel[b, b * (P // B) : (b + 1) * (P // B)] = 1.0
    sel[B, :] = 1.0
    in_maps = []
    for i in range(N_CORES):
        sl = slice(i * JC, (i + 1) * JC)
        in_maps.append(
            {
                "vtb": vt,
                "wob": _chunked(woT[:, sl]),
                "bob": bob[sl].reshape(1, JC),
                "selb": sel,
            }
        )
    return in_maps


_NC_CACHE = None


def get_ncs():
    global _NC_CACHE
    if _NC_CACHE is None:
        _NC_CACHE = (build_nc_a(), build_nc_b())
    return _NC_CACHE


def assemble_output(results_b):
    """Per-core (P, LINE) bf16 blocks -> full (B, S, D) fp32."""
    shards = [r["out"].reshape(B, S, JC) for r in results_b]
    return np.concatenate(shards, axis=-1).astype(np.float32)


def kernel(**inputs):
    nc_a, nc_b = get_ncs()
    cores = list(range(N_CORES))

    res_a = run_bass_kernel_spmd(
        nc_a,
        make_in_maps_a(inputs["condition"], inputs["Wv"], inputs["bv"]),
        core_ids=cores,
    )
    v_bf = np.concatenate([r["v_s"] for r in res_a.results], axis=1)

    res_b = run_bass_kernel_spmd(
        nc_b,
        make_in_maps_b(v_bf, inputs["Wo"], inputs["bo"]),
        core_ids=cores,
    )
    return assemble_output(res_b.results)


# revision 6
# speedup vs baseline: 2.5706x; 1.6673x over previous
"""Trainium2 Bass kernel for CrossAttentionConditionInjection.

Math note: in the reference, K and V are projections of a single per-batch
condition vector broadcast identically across all S key positions.  The
attention scores are therefore constant along the softmax axis, softmax is
exactly uniform (1/S each), and the attention output is the mean of S
identical V rows, i.e. V itself.  The whole module collapses exactly to

    out[b, s, :] = (condition[b] @ Wv.T + bv) @ Wo.T + bo      (for every s)

independent of hidden_states / Wq / bq / Wk / bk.  (S = 1024 is a power of
two, so even the fp32 softmax-average path is bit-exact against this.)

Sharding (follows the spec hint "tensor-parallel ... shard cond_to_v output
dim and out_proj input dim"):  core i owns channel block i of the v
projection.  It computes v_i = condition @ Wv.T[:, sh_i] + bv[sh_i] and then
the out-projection partial  po_i = v_i @ Wo.T[sh_i, :] (+ bo on core 0), a
(4, 2048) fp32 partial sum.  The unshard step for contraction sharding is a
sum over cores, done on host between launches (a device AllReduce costs
~80us in this runtime).  All multiply-adds run on device in one NEFF
(launch XA).

Output materialization: out[b, s, :] is the same row for every s.  Two
modes:
  MODE "iii": host tiles the summed (4, 2048) row over S (pure layout).
  MODE "x":   a second NEFF (XB) materializes the full (4, 1024, 2048)
              output on device: one selector matmul broadcasts the 4 rows
              to 128 partitions, log2 DVE doublings build 4 KB lines, and
              two HWDGE DMAs with stride-0 source APs replicate to the
              (128, 8192) bf16 output block per core.

Device-perf notes (vs the 80.4us first working version):
  - bf16 weights/activations (4x PE throughput, half the DMA bytes);
    fp32 PSUM and fp32 partial sums.  Measured rel err ~4e-3 vs the 2e-2
    gate.
  - weights staged host-side into partition-major [128, KT*...] bf16 so
    every DMA moves 4-8 KB contiguous per partition.
  - bulk DMAs split across both HWDGE rings (sync + scalar engines).
"""

import numpy as np
import ml_dtypes

import concourse.bass as bass
import concourse.mybir as mybir
import concourse.tile as tile
from concourse import bacc
from concourse.bass_utils import run_bass_kernel_spmd

B = 4
S = 1024
D = 2048
N_CORES = 8
JC = D // N_CORES  # 256 v-channels per core
P = 128
KT = D // P  # 16 k-chunks for matmul 1
MT = JC // P  # 2 k-chunks for matmul 2
NB = 4  # psum banks for the (4, 2048) partial (n=512 each)
FP = mybir.dt.float32
BF = mybir.dt.bfloat16
NPBF = ml_dtypes.bfloat16

RPP = (B * S) // P  # 32 replicated rows per partition in the output block
LINE = RPP * JC  # 8192 bf16 elems per output partition line
REP_N = 2048  # replicated elems built in SBUF (4 KB lines); DMA repeats 4x

MODE = "iii"  # "iii": host tiles the S-broadcast; "x": device writes full out


def _new_nc():
    return bacc.Bacc(
        "TRN2",
        target_bir_lowering=False,
        debug=False,
        enable_asserts=False,
        num_devices=N_CORES,
    )


def _chunked(a2d, p=P):
    """(K, n) -> [p, (K//p)*n] bf16; chunk t holds rows t*p..t*p+p-1."""
    k, n = a2d.shape
    t = k // p
    return np.ascontiguousarray(
        a2d.astype(NPBF).reshape(t, p, n).transpose(1, 0, 2).reshape(p, t * n)
    )


def build_nc_xa():
    """Per core i: po_i = (cond @ WvT[:, sh_i] + bv_i) @ WoT[sh_i, :] (+ bo)."""
    nc = _new_nc()
    ct_d = nc.dram_tensor("ctb", [P, KT * B], BF, kind="ExternalInput").ap()
    wv_d = nc.dram_tensor("wvb", [P, KT * JC], BF, kind="ExternalInput").ap()
    bv_d = nc.dram_tensor("bvb", [1, JC], BF, kind="ExternalInput").ap()
    wo_d = nc.dram_tensor("wor", [P, MT * D], BF, kind="ExternalInput").ap()
    bo_d = nc.dram_tensor("bob", [1, D], BF, kind="ExternalInput").ap()
    id_d = nc.dram_tensor("id4", [B, B], BF, kind="ExternalInput").ap()
    po_d = nc.dram_tensor("po", [B, D], FP, kind="ExternalOutput").ap()

    H = KT // 2
    with tile.TileContext(nc) as tc:
        with (
            tc.tile_pool(name="work", bufs=1) as work,
            tc.tile_pool(name="pv", bufs=1, space="PSUM") as pv_pool,
            tc.tile_pool(name="pt", bufs=2, space="PSUM") as pt_pool,
            tc.tile_pool(name="pp", bufs=1, space="PSUM") as pp_pool,
        ):
            wv_sb = work.tile([P, KT, JC], BF)
            ct_sb = work.tile([P, KT, B], BF)
            bv_sb = work.tile([1, JC], BF)
            wo_sb = work.tile([P, MT, D], BF)
            bo_sb = work.tile([1, D], BF)
            id_sb = work.tile([B, B], BF)
            ones_sb = work.tile([1, B], BF)
            v_sb = work.tile([B, JC], BF)
            vt_sb = work.tile([P, MT, B], BF)
            po_sb = work.tile([B, D], FP)

            # sync ring: ct then the two wv halves (mm1-critical)
            nc.sync.dma_start(ct_sb[:, :, :], ct_d.rearrange("p (t b) -> p t b", t=KT))
            nc.sync.dma_start(
                wv_sb[:, 0:H, :],
                wv_d[:, 0 : H * JC].rearrange("p (t j) -> p t j", t=H),
            )
            nc.sync.dma_start(
                wv_sb[:, H:KT, :],
                wv_d[:, H * JC : KT * JC].rearrange("p (t j) -> p t j", t=H),
            )
            # scalar ring: small tensors then the two wo halves
            nc.scalar.dma_start(bv_sb[:, :], bv_d[:, :])
            nc.scalar.dma_start(id_sb[:, :], id_d[:, :])
            nc.scalar.dma_start(
                wo_sb[:, 0, :], wo_d[:, 0:D].rearrange("p (t j) -> p t j", t=1)
            )
            nc.scalar.dma_start(
                wo_sb[:, 1, :], wo_d[:, D : 2 * D].rearrange("p (t j) -> p t j", t=1)
            )
            nc.scalar.dma_start(bo_sb[:, :], bo_d[:, :])
            nc.vector.memset(ones_sb[:, :], 1.0)

            # mm1: v = cond @ WvT[:, sh] + bv  -> (4, 256) fp32
            pv = pv_pool.tile([B, JC], FP)
            for t in range(KT):
                nc.tensor.matmul(
                    pv[:, :],
                    ct_sb[:, t, :],
                    wv_sb[:, t, :],
                    start=(t == 0),
                    stop=False,
                )
            nc.tensor.matmul(
                pv[:, :], ones_sb[:, :], bv_sb[:, :], start=False, stop=True
            )
            nc.vector.tensor_copy(v_sb[:, :], pv[:, :])

            # transpose v -> vT chunks [128, 4] for mm2's lhsT
            for g in range(MT):
                pt = pt_pool.tile([P, B], BF)
                nc.tensor.transpose(
                    pt[:, :], v_sb[:, g * P : (g + 1) * P], id_sb[:, :]
                )
                nc.vector.tensor_copy(vt_sb[:, g, :], pt[:, :])

            # mm2: po = v @ WoT[sh, :] (+ bo), 4 psum banks of n=512
            pps = [
                pp_pool.tile([B, D // NB], FP, name=f"pp{k}", tag=f"pp{k}")
                for k in range(NB)
            ]
            for g in range(MT):
                for k in range(NB):
                    nc.tensor.matmul(
                        pps[k][:, :],
                        vt_sb[:, g, :],
                        wo_sb[:, g, k * (D // NB) : (k + 1) * (D // NB)],
                        start=(g == 0),
                        stop=False,
                    )
            for k in range(NB):
                nc.tensor.matmul(
                    pps[k][:, :],
                    ones_sb[:, :],
                    bo_sb[:, k * (D // NB) : (k + 1) * (D // NB)],
                    start=False,
                    stop=True,
                )
                nc.vector.tensor_copy(
                    po_sb[:, k * (D // NB) : (k + 1) * (D // NB)], pps[k][:, :]
                )
            nc.sync.dma_start(po_d[:, :], po_sb[:, :])

    nc.compile()
    return nc


def build_nc_xb():
    """Per core i: broadcast the final (4, 256) row shard to (4, 1024, 256)."""
    nc = _new_nc()
    rb_d = nc.dram_tensor("rbb", [B + 1, JC], BF, kind="ExternalInput").ap()
    sel_d = nc.dram_tensor("selb", [B + 1, P], BF, kind="ExternalInput").ap()
    out_d = nc.dram_tensor("out", [P, LINE], BF, kind="ExternalOutput").ap()

    with tile.TileContext(nc) as tc:
        with (
            tc.tile_pool(name="work", bufs=1) as work,
            tc.tile_pool(name="pb", bufs=1, space="PSUM") as pb_pool,
        ):
            rb_sb = work.tile([B + 1, JC], BF)
            sel_sb = work.tile([B + 1, P], BF)
            rep_sb = work.tile([P, REP_N], BF)

            nc.sync.dma_start(rb_sb[:, :], rb_d[:, :])
            nc.scalar.dma_start(sel_sb[:, :], sel_d[:, :])

            pb = pb_pool.tile([P, JC], FP)
            nc.tensor.matmul(
                pb[:, :], sel_sb[:, :], rb_sb[:, :], start=True, stop=True
            )
            nc.vector.tensor_copy(rep_sb[:, 0:JC], pb[:, :])
            n = JC
            while n < REP_N:
                nc.vector.tensor_copy(rep_sb[:, n : 2 * n], rep_sb[:, 0:n])
                n *= 2

            # two stride-0 replicating writes, one per HWDGE ring
            rep_half = LINE // 2 // REP_N  # repeats per half-line
            src = rep_sb[:, :].unsqueeze(1).broadcast_to([P, rep_half, REP_N])
            nc.sync.dma_start(
                out_d[:, 0 : LINE // 2].rearrange("p (r n) -> p r n", r=rep_half),
                src,
            )
            nc.scalar.dma_start(
                out_d[:, LINE // 2 : LINE].rearrange("p (r n) -> p r n", r=rep_half),
                src,
            )

    nc.compile()
    return nc


def make_in_maps_a(condition, Wv, bv, Wo, bo):
    ct = _chunked(np.asarray(condition, dtype=np.float32).T)  # [P, KT*B]
    wvT = np.asarray(Wv, dtype=np.float32).T
    woT = np.asarray(Wo, dtype=np.float32).T
    bvb = np.asarray(bv, dtype=np.float32).astype(NPBF)
    bob = np.asarray(bo, dtype=np.float32).astype(NPBF)
    bo0 = np.zeros((1, D), dtype=NPBF)
    id4 = np.eye(B, dtype=NPBF)
    in_maps = []
    for i in range(N_CORES):
        sl = slice(i * JC, (i + 1) * JC)
        in_maps.append(
            {
                "ctb": ct,
                "wvb": _chunked(wvT[:, sl]),
                "bvb": bvb[sl].reshape(1, JC),
                "wor": _chunked(woT[sl, :]),
                "bob": bob.reshape(1, D) if i == 0 else bo0,
                "id4": id4,
            }
        )
    return in_maps


def make_in_maps_b(out_row):
    """out_row: (B, D) fp32 final row (bias included)."""
    rb = np.zeros((B + 1, JC * N_CORES), dtype=NPBF)
    rb[0:B, :] = out_row.astype(NPBF)
    sel = np.zeros((B + 1, P), dtype=NPBF)
    for b in range(B):
        sel[b, b * (P // B) : (b + 1) * (P // B)] = 1.0
    in_maps = []
    for i in range(N_CORES):
        sl = slice(i * JC, (i + 1) * JC)
        in_maps.append({"rbb": np.ascontiguousarray(rb[:, sl]), "selb": sel})
    return in_maps


_NC_CACHE = None


def get_ncs():
    global _NC_CACHE
    if _NC_CACHE is None:
        nc_a = build_nc_xa()
        nc_b = build_nc_xb() if MODE == "x" else None
        _NC_CACHE = (nc_a, nc_b)
    return _NC_CACHE


def sum_partials(results_a):
    out_row = np.zeros((B, D), dtype=np.float32)
    for r in results_a:
        out_row += r["po"]
    return out_row


def assemble_output_x(results_b):
    shards = [r["out"].reshape(B, S, JC) for r in results_b]
    return np.concatenate(shards, axis=-1).astype(np.float32)


def assemble_output_iii(out_row):
    out = np.empty((B, S, D), dtype=np.float32)
    out[:] = out_row[:, None, :]
    return out


def kernel(**inputs):
    nc_a, nc_b = get_ncs()
    cores = list(range(N_CORES))

    res_a = run_bass_kernel_spmd(
        nc_a,
        make_in_maps_a(
            inputs["condition"], inputs["Wv"], inputs["bv"], inputs["Wo"], inputs["bo"]
        ),
        core_ids=cores,
    )
    out_row = sum_partials(res_a.results)

    if MODE == "iii":
        return assemble_output_iii(out_row)

    res_b = run_bass_kernel_spmd(nc_b, make_in_maps_b(out_row), core_ids=cores)
    return assemble_output_x(res_b.results)
